# revision 1
# baseline (speedup 1.0000x reference)
"""Bass/Trainium2 kernel for nn_Attention (ragged masked-softmax attention).

Math (per batch b with valid length L):
    c_b      = W_h @ hidden[:, b] + b_attn                  # [2H], W_h = W_attn[:, :H]
    e[s, :]  = tanh(W_e @ x_s + c_b)                        # W_e = W_attn[:, H:]
    score[s] = w_v . e[s, :] + b_v            (s < L)
    energy   = softmax(score[:L]);  context = energy @ X[:L]

Device strategy: the ragged work is split into fixed 256-position chunks
("units", 72 total for the graded lengths), distributed evenly over 8 cores
(one identical static SPMD program; per-core behavior differs only through
data).  Each unit produces flash-softmax partials (m, Z, ctx) which the host
merges exactly.  Matmul operands are fp16 (full-rate on the PE, fp32 PSUM
accumulation); softmax is fp32.
"""

import numpy as np

import concourse.bass as bass
import concourse.mybir as mybir
import concourse.tile as tile
from concourse import bacc
from concourse.bass_utils import run_bass_kernel_spmd

B, S, H = 16, 2048, 1024
H2 = 2 * H            # 2048 output features / encoder dim
CHUNK = 256           # sequence positions per work unit
N_CORES = 8
FB = H2 // 128        # 16 f-blocks of the contraction dim (encoder features)
OB = H2 // 128        # 16 o-blocks of the output features
HB = H // 128         # 8 h-blocks of the hidden contraction
NEG = -30000.0        # masked-score offset (exp underflows to exactly 0)

F16 = mybir.dt.float16
F32 = mybir.dt.float32


def build_program(nchunk: int, nhb: int = HB + 1):
    nc = bacc.Bacc()

    xt_ext = nc.declare_dram_parameter("xt", [nchunk, 128, FB, CHUNK], F16, isOutput=False)
    xn_ext = nc.declare_dram_parameter("xn", [nchunk, 128, CHUNK // 128, H2], F16, isOutput=False)
    mask_ext = nc.declare_dram_parameter("mask", [nchunk, CHUNK], F32, isOutput=False)
    hu_ext = nc.declare_dram_parameter("hu", [128, nhb, nchunk], F16, isOutput=False)
    # weights are staged o-block-major so compute can start after ~1MB of DMA
    wet_ext = nc.declare_dram_parameter("wet", [OB, 128, FB, 128], F16, isOutput=False)
    wht_ext = nc.declare_dram_parameter("wht", [OB, 128, nhb, 128], F16, isOutput=False)
    wv_ext = nc.declare_dram_parameter("wv", [128, OB], F16, isOutput=False)
    ctx_out = nc.declare_dram_parameter("out_ctx", [nchunk, H2], F32, isOutput=True)
    mz_out = nc.declare_dram_parameter("out_mz", [nchunk, 2], F32, isOutput=True)

    SB = CHUNK // 128   # s-blocks per unit for the context matmul
    DQ = H2 // 512      # 512-wide output quarters for the context matmul

    from contextlib import ExitStack
    with tile.TileContext(nc) as tc, ExitStack() as stk:
        singles = stk.enter_context(tc.tile_pool(name="singles", bufs=1))
        xtp = stk.enter_context(tc.tile_pool(name="xtp", bufs=2))
        xnp = stk.enter_context(tc.tile_pool(name="xnp", bufs=3))
        tp = stk.enter_context(tc.tile_pool(name="tp", bufs=2))
        smalls = stk.enter_context(tc.tile_pool(name="smalls", bufs=3))
        eps = stk.enter_context(tc.tile_pool(name="eps", bufs=3, space="PSUM"))
        sps = stk.enter_context(tc.tile_pool(name="sps", bufs=2, space="PSUM"))
        cps = stk.enter_context(tc.tile_pool(name="cps", bufs=2, space="PSUM"))

        # resident weights as one tile per o-block (fine-grained DMA deps so
        # the PE can start as soon as the first o-block's weights land)
        wet_sb = []
        wht_sb = []
        hu_sb = singles.tile([128, nhb, nchunk], F16)
        wv_sb = singles.tile([128, OB], F16)
        mask_sb = singles.tile([1, nchunk, CHUNK], F32)
        xt0_sb = xtp.tile([128, FB, CHUNK], F16, tag="xt")
        for ob in range(OB):
            if ob == 0:
                # the very first PE work is C(0) = wht0 x hu: land those first
                nc.sync.dma_start(out=hu_sb[:], in_=hu_ext[:])
            w2 = singles.tile([128, nhb, 128], F16, tag=f"wht{ob}")
            nc.sync.dma_start(out=w2[:], in_=wht_ext[ob])
            w1 = singles.tile([128, FB, 128], F16, tag=f"wet{ob}")
            nc.sync.dma_start(out=w1[:], in_=wet_ext[ob])
            wet_sb.append(w1)
            wht_sb.append(w2)
            if ob == 0:
                nc.sync.dma_start(out=wv_sb[:], in_=wv_ext[:])
                nc.sync.dma_start(out=mask_sb[0:1, :, :], in_=mask_ext[:])
                nc.sync.dma_start(out=xt0_sb[:], in_=xt_ext[0])
        mz_all = singles.tile([1, nchunk, 2], F32)
        ident_sb = singles.tile([1, 1], F16)
        nc.vector.memset(ident_sb[:], 1.0)

        # per-unit bias columns: c[o, i] = sum_h W_h[o, h] hu[h, i] (+ b_attn
        # row).  Emitted lazily inside unit 0's ob loop so each C(ob) group
        # sits right before the e-group that unblocks tanh(ob).
        c_sb = [None] * OB

        def emit_c(ob):
            c_ps = cps.tile([128, nchunk], F32, tag="cps")
            for jh in range(nhb):
                nc.tensor.matmul(
                    c_ps[:],
                    lhsT=wht_sb[ob][:, jh, :],
                    rhs=hu_sb[:, jh, :],
                    start=(jh == 0), stop=(jh == nhb - 1),
                )
            c1 = singles.tile([128, nchunk], F32, tag=f"c{ob}")
            nc.vector.tensor_copy(out=c1[:], in_=c_ps[:])
            c_sb[ob] = c1

        def emit_xn_dma(p):
            i, xn_sb = p[0], p[2]
            nc.sync.dma_start(out=xn_sb[:], in_=xn_ext[i])

        def emit_ctx(p):
            i, pt_sb, xn_sb = p[0], p[1], p[2]
            ctx_sb = smalls.tile([1, H2], F32, tag="ctx")
            for dq in range(DQ):
                ctx_ps = cps.tile([1, 512], F32, tag="cps")
                for sb in range(SB):
                    nc.tensor.matmul(
                        ctx_ps[:],
                        lhsT=pt_sb[:, sb:sb + 1],
                        rhs=xn_sb[:, sb, dq * 512:(dq + 1) * 512],
                        start=(sb == 0), stop=(sb == SB - 1),
                    )
                if dq % 2 == 0:
                    nc.vector.tensor_copy(out=ctx_sb[0:1, dq * 512:(dq + 1) * 512], in_=ctx_ps[:])
                else:
                    nc.scalar.copy(out=ctx_sb[0:1, dq * 512:(dq + 1) * 512], in_=ctx_ps[:])
            nc.sync.dma_start(out=ctx_out[i], in_=ctx_sb[0:1, :])

        def emit_egroup(i, xt_sb, t_sb, ob):
            if c_sb[ob] is None:
                emit_c(ob)
            e_ps = eps.tile([128, CHUNK], F32, tag="e")
            for fb in range(FB):
                nc.tensor.matmul(
                    e_ps[:],
                    lhsT=wet_sb[ob][:, fb, :],
                    rhs=xt_sb[:, fb, :],
                    start=(fb == 0), stop=(fb == FB - 1),
                )
            nc.scalar.activation(
                out=t_sb[:, ob, :], in_=e_ps[:],
                func=mybir.ActivationFunctionType.Tanh,
                bias=c_sb[ob][:, i:i + 1], scale=1.0,
            )

        def emit_scores(i, t_sb):
            # scores[s] = sum_o w_v[o] t[o, s] -> 4 partial rows (PE column
            # groups run concurrently; tile_position derives from the slices)
            s_ps = sps.tile([128, CHUNK], F32, tag="s", bufs=1)
            for r in range(OB // 4):
                for j in range(4):
                    ob = r * 4 + j
                    nc.tensor.matmul(
                        s_ps[32 * j:32 * j + 1, :],
                        lhsT=wv_sb[:, ob:ob + 1],
                        rhs=t_sb[:, ob, :],
                        start=(r == 0), stop=(r == OB // 4 - 1),
                        tile_position=(0, 32 * j),
                    )
            return s_ps

        def emit_softmax(i, s_ps):
            # masked softmax partials: fold the 4 partial rows + mask
            # (DVE may read at most one PSUM operand per op -> serial chain)
            acc_sb = []
            for j in range(4):
                prev = mask_sb[0:1, i, :] if j == 0 else acc_sb[-1][:]
                a = smalls.tile([1, CHUNK], F32, tag=f"fold{j}")
                nc.vector.tensor_tensor(
                    out=a[:], in0=s_ps[32 * j:32 * j + 1, :], in1=prev,
                    op=mybir.AluOpType.add,
                )
                acc_sb.append(a)
            sc_sb = acc_sb[-1]
            negm_sb = smalls.tile([1, 1], F32, tag="negm")
            nc.vector.tensor_reduce(
                out=negm_sb[:], in_=sc_sb[:],
                axis=mybir.AxisListType.X, op=mybir.AluOpType.max, negate=True,
            )
            p_sb = smalls.tile([1, CHUNK], F16, tag="p")
            z_sb = smalls.tile([1, 1], F32, tag="z")
            nc.scalar.activation(
                out=p_sb[:], in_=sc_sb[:],
                func=mybir.ActivationFunctionType.Exp,
                bias=negm_sb[0:1, :], scale=1.0, accum_out=z_sb[:],
            )
            nc.vector.tensor_copy(out=mz_all[0:1, i, 0:1], in_=negm_sb[:])
            nc.vector.tensor_copy(out=mz_all[0:1, i, 1:2], in_=z_sb[:])
            xn_sb = xnp.tile([128, SB, H2], F16, tag="xn")
            return [i, p_sb, xn_sb]

        def emit_pt(p):
            # p row -> column layout [128, SB] via PE transpose.  Deferred to
            # the NEXT unit's PE stream (after its e-groups) so the transpose
            # never waits on the softmax chain.
            i, p_sb, xn_sb = p
            pt_sb = smalls.tile([128, SB], F16, tag="pt")
            for sb in range(SB):
                t_ps = sps.tile([128, 1], F16, tag="tp", bufs=2)
                nc.tensor.transpose(
                    t_ps[:], p_sb[0:1, sb * 128:(sb + 1) * 128], ident_sb[:])
                nc.vector.tensor_copy(out=pt_sb[:, sb:sb + 1], in_=t_ps[:])
            p[1] = pt_sb

        pending = []
        for i in range(nchunk):
            if i == 0:
                xt_sb = xt0_sb
            else:
                xt_sb = xtp.tile([128, FB, CHUNK], F16, tag="xt")
                nc.sync.dma_start(out=xt_sb[:], in_=xt_ext[i])
            if pending:
                emit_xn_dma(pending[-1])  # queued behind this unit's xt

            t_sb = tp.tile([128, OB, CHUNK], F16, tag="t")
            for ob in range(OB):
                emit_egroup(i, xt_sb, t_sb, ob)

            for p in pending:
                emit_pt(p)
            s_ps = emit_scores(i, t_sb)
            while pending:
                emit_ctx(pending.pop(0))
            pending.append(emit_softmax(i, s_ps))

        if pending:
            emit_xn_dma(pending[-1])
        for p in pending:
            emit_pt(p)
        while pending:
            emit_ctx(pending.pop(0))
        nc.sync.dma_start(out=mz_out[:], in_=mz_all[0:1, :, :])

    nc.compile()
    return nc


def kernel(encoder_out, hidden, W_attn, b_attn, w_v, b_v, lengths):
    encoder_out = np.asarray(encoder_out)
    hidden = np.asarray(hidden)
    W_attn = np.asarray(W_attn)
    b_attn = np.asarray(b_attn)
    w_v = np.asarray(w_v)
    b_v = np.asarray(b_v)
    lengths = np.asarray(lengths)

    # ---- host-side work-unit schedule from the runtime lengths ----
    units = []  # (batch, s0, valid)
    for b in range(B):
        L = int(lengths[b])
        for s0 in range(0, L, CHUNK):
            units.append((b, s0, min(CHUNK, L - s0)))
    nchunk = max(1, (len(units) + N_CORES - 1) // N_CORES)

    # ---- replicated weight layouts (fp16), o-block-major ----
    # wet[ob, p, fb, q] = W_e^T[fb*128+p, ob*128+q] = W_attn[ob*128+q, H + fb*128+p]
    wet = np.ascontiguousarray(
        W_attn[:, H:].T.reshape(FB, 128, OB, 128).transpose(2, 1, 0, 3)
    ).astype(np.float16)
    # wht[ob, p, jh, q]: blocks 0..HB-1 of W_h^T; an extra block whose row
    # p=0 carries b_attn is appended only when b_attn is nonzero
    nhb = HB + 1 if np.any(b_attn) else HB
    wht_aug = np.zeros((nhb * 128, H2), np.float32)
    wht_aug[:H] = W_attn[:, :H].T
    if nhb > HB:
        wht_aug[H] = b_attn
    wht = np.ascontiguousarray(
        wht_aug.reshape(nhb, 128, OB, 128).transpose(2, 1, 0, 3)
    ).astype(np.float16)
    wv = np.ascontiguousarray(w_v[0].reshape(OB, 128).T).astype(np.float16)

    # ---- per-core gathered inputs ----
    in_maps = []
    slot_of = []  # per real unit: (core, slot)
    x16 = encoder_out.astype(np.float16)
    for c in range(N_CORES):
        cu = units[c * nchunk:(c + 1) * nchunk]
        xt = np.zeros((nchunk, 128, FB, CHUNK), np.float16)
        xn = np.zeros((nchunk, 128, CHUNK // 128, H2), np.float16)
        mask = np.full((nchunk, CHUNK), NEG + float(b_v[0]), np.float32)
        hu = np.zeros((128, nhb, nchunk), np.float16)
        if nhb > HB:
            hu[0, HB, :] = 1.0
        for slot, (b, s0, v) in enumerate(cu):
            chunk = x16[b, s0:s0 + v, :]                      # [v, 2048]
            xt[slot, :, :, :v] = chunk.T.reshape(FB, 128, v).transpose(1, 0, 2)
            # xn[slot, p, sb, d] = chunk[sb*128 + p, d]
            full = np.zeros((CHUNK, H2), np.float16)
            full[:v] = chunk
            xn[slot] = full.reshape(CHUNK // 128, 128, H2).transpose(1, 0, 2)
            mask[slot, :v] = float(b_v[0])
            hu[:, :HB, slot] = hidden[:, b].reshape(HB, 128).T
            slot_of.append((c, slot))
        in_maps.append(dict(
            xt=xt, xn=xn, mask=mask, hu=hu,
            wet=wet, wht=wht, wv=wv,
        ))

    nc = build_program(nchunk, nhb)

    def run_once():
        res = run_bass_kernel_spmd(nc, in_maps, core_ids=list(range(N_CORES)))
        negm = np.stack([res.results[c]["out_mz"][:, 0] for c in range(N_CORES)])
        zz = np.stack([res.results[c]["out_mz"][:, 1] for c in range(N_CORES)])
        ctx = np.stack([res.results[c]["out_ctx"] for c in range(N_CORES)])
        return negm, zz, ctx

    def merge(parts):
        negm, zz, ctx = parts
        # ---- exact flash-softmax merge on host ----
        out = np.zeros((B, H2), np.float32)
        ok = np.isfinite(negm).all() and np.isfinite(zz).all() and np.isfinite(ctx).all()
        for b in range(B):
            idxs = [slot_of[k] for k, (ub, _, _) in enumerate(units) if ub == b]
            ms = np.array([-float(negm[c, s]) for c, s in idxs])
            m = ms.max()
            w = np.exp(ms - m)
            Z = float(sum(wi * float(zz[c, s]) for wi, (c, s) in zip(w, idxs)))
            if not (Z > 0):
                ok = False
                Z = 1.0
            acc = np.zeros(H2, np.float64)
            for wi, (c, s) in zip(w, idxs):
                acc += wi * ctx[c, s].astype(np.float64)
            out[b] = (acc / Z).astype(np.float32)
        # context rows are convex combinations of encoder_out rows
        ok = ok and np.isfinite(out).all() and np.abs(out).max() < 50.0
        return out, ok

    out, ok = merge(run_once())
    if not ok:  # one retry on gross corruption
        out, ok = merge(run_once())
    return out



# revision 8
# speedup vs baseline: 1.5923x; 1.5923x over previous
"""Bass/Trainium2 kernel for nn_Attention (ragged masked-softmax attention).

Math (per batch b with valid length L):
    c_b      = W_h @ hidden[:, b] + b_attn                  # [2H], W_h = W_attn[:, :H]
    e[s, :]  = tanh(W_e @ x_s + c_b)                        # W_e = W_attn[:, H:]
    score[s] = w_v . e[s, :] + b_v            (s < L)
    energy   = softmax(score[:L]);  context = energy @ X[:L]

Device strategy: ragged work split into 256-position chunks distributed over
8 cores (one static SPMD program).  The dominant e-matmul runs in fp8e4 with
perf_mode=DoubleRow (K=256 per instruction, 2x PE throughput); chunks are
processed in PAIRS so each DoubleRow matmul streams 512 columns and the
LDWEIGHTS pipe hides under the stream.  W_e is pre-scaled x512 and X x16 so
fp8 values stay in the normal range; the 1/8192 descale folds into the tanh
activation's scale.  The fp8 quantization error is compensated host-side: a
linearized score correction  C[s] = sum_o w_v[o] kappa[o,b] (Delta e)[o,s]
(exactly decomposable into host matvecs against the known quantization
residuals) is folded into the per-position mask row.  Softmax + the context
matmul stay fp16/fp32.  Each chunk produces flash-softmax partials
(m, Z, ctx) which the host merges exactly.
"""

import numpy as np
import ml_dtypes

import concourse.bass as bass
import concourse.mybir as mybir
import concourse.tile as tile
from concourse import bacc
from concourse.bass_utils import run_bass_kernel_spmd

B, S, H = 16, 2048, 1024
H2 = 2 * H            # 2048 output features / encoder dim
CHUNK = 256           # sequence positions per work unit
N_CORES = 8
FB2 = H2 // 256       # 8 fp8 DoubleRow f-blocks (256 features each)
OB = H2 // 128        # 16 o-blocks of the output features
HB = H // 128         # 8 h-blocks of the hidden contraction
NEG = -30000.0        # masked-score offset (exp underflows to exactly 0)
ALPHA = 512.0         # W_e fp8 pre-scale
BETA = 16.0           # X fp8 pre-scale
ISCL = 1.0 / (ALPHA * BETA)

F8 = mybir.dt.float8e4
F16 = mybir.dt.float16
F32 = mybir.dt.float32
NP_F8 = ml_dtypes.float8_e4m3   # TRN fp8_exp4 (max normal +-240)


def build_program(nchunk: int, nhb: int = HB + 1):
    nc = bacc.Bacc()

    npairs = nchunk // 2
    nsingle = nchunk % 2
    # groups: list of (dram_index, [chunk indices])
    groups = [(k, [2 * k, 2 * k + 1]) for k in range(npairs)]
    if nsingle:
        groups.append((0, [nchunk - 1]))

    if npairs:
        xtp_ext = nc.declare_dram_parameter(
            "xtp", [npairs, 128, FB2, 2, 2 * CHUNK], F8, isOutput=False)
    if nsingle:
        xts_ext = nc.declare_dram_parameter(
            "xts", [1, 128, FB2, 2, CHUNK], F8, isOutput=False)
    xn_ext = nc.declare_dram_parameter("xn", [nchunk, 128, CHUNK // 128, H2], F16, isOutput=False)
    mask_ext = nc.declare_dram_parameter("mask", [nchunk, CHUNK], F32, isOutput=False)
    hu_ext = nc.declare_dram_parameter("hu", [128, nhb, nchunk], F16, isOutput=False)
    # weights are staged o-block-major so compute can start after ~1MB of DMA
    wet_ext = nc.declare_dram_parameter("wet", [OB, 128, FB2, 2, 128], F8, isOutput=False)
    wht_ext = nc.declare_dram_parameter("wht", [OB, 128, nhb, 128], F16, isOutput=False)
    wv_ext = nc.declare_dram_parameter("wv", [128, OB], F16, isOutput=False)
    ctx_out = nc.declare_dram_parameter("out_ctx", [nchunk, H2], F32, isOutput=True)
    mz_out = nc.declare_dram_parameter("out_mz", [nchunk, 2], F32, isOutput=True)

    SB = CHUNK // 128   # s-blocks per unit for the context matmul
    DQ = H2 // 512      # 512-wide output quarters for the context matmul
    DR = mybir.MatmulPerfMode.DoubleRow

    from contextlib import ExitStack
    with tile.TileContext(nc) as tc, ExitStack() as stk:
        singles = stk.enter_context(tc.tile_pool(name="singles", bufs=1))
        xtp = stk.enter_context(tc.tile_pool(name="xtp", bufs=2))
        xnp = stk.enter_context(tc.tile_pool(name="xnp", bufs=3))
        tp = stk.enter_context(tc.tile_pool(name="tp", bufs=4))
        smalls = stk.enter_context(tc.tile_pool(name="smalls", bufs=3))
        eps = stk.enter_context(tc.tile_pool(name="eps", bufs=2, space="PSUM"))
        sps = stk.enter_context(tc.tile_pool(name="sps", bufs=2, space="PSUM"))
        cps = stk.enter_context(tc.tile_pool(name="cps", bufs=2, space="PSUM"))

        g0_w = 2 * CHUNK if npairs else CHUNK

        # resident weights as one tile per o-block (fine-grained DMA deps so
        # the PE can start as soon as the first o-block's weights land)
        wet_sb = []
        wht_sb = []
        hu_sb = singles.tile([128, nhb, nchunk], F16)
        wv_sb = singles.tile([128, OB], F16)
        mask_sb = singles.tile([1, nchunk, CHUNK], F32)
        xt0_sb = xtp.tile([128, FB2, 2, g0_w], F8, tag="xt")
        for ob in range(OB):
            if ob == 0:
                # the very first PE work is C(0) = wht0 x hu: land those first
                nc.sync.dma_start(out=hu_sb[:], in_=hu_ext[:])
            w2 = singles.tile([128, nhb, 128], F16, tag=f"wht{ob}")
            nc.sync.dma_start(out=w2[:], in_=wht_ext[ob])
            w1 = singles.tile([128, FB2, 2, 128], F8, tag=f"wet{ob}")
            nc.sync.dma_start(out=w1[:], in_=wet_ext[ob])
            wet_sb.append(w1)
            wht_sb.append(w2)
            if ob == 0:
                nc.sync.dma_start(out=wv_sb[:], in_=wv_ext[:])
                nc.sync.dma_start(out=mask_sb[0:1, :, :], in_=mask_ext[:])
                if npairs:
                    nc.sync.dma_start(out=xt0_sb[:], in_=xtp_ext[0])
                else:
                    nc.sync.dma_start(out=xt0_sb[:], in_=xts_ext[0])
        mz_all = singles.tile([1, nchunk, 2], F32)
        ident_sb = singles.tile([1, 1], F16)
        nc.vector.memset(ident_sb[:], 1.0)

        # per-unit bias columns: c[o, i] = sum_h W_h[o, h] hu[h, i] (+ b_attn
        # row).  Emitted lazily inside group 0's ob loop so each C(ob) group
        # sits right before the e-group that unblocks tanh(ob).
        c_sb = [None] * OB

        def emit_c(ob):
            c_ps = cps.tile([128, nchunk], F32, tag="cps")
            for jh in range(nhb):
                nc.tensor.matmul(
                    c_ps[:],
                    lhsT=wht_sb[ob][:, jh, :],
                    rhs=hu_sb[:, jh, :],
                    start=(jh == 0), stop=(jh == nhb - 1),
                )
            c1 = singles.tile([128, nchunk], F32, tag=f"c{ob}")
            nc.vector.tensor_copy(out=c1[:], in_=c_ps[:])
            c_sb[ob] = c1

        def emit_xn_dma(p):
            i, xn_sb = p[0], p[2]
            if not p[4]:
                p[4] = True
                nc.sync.dma_start(out=xn_sb[:], in_=xn_ext[i])

        def emit_ctx(p):
            i, pt_sb, xn_sb = p[0], p[1], p[2]
            ctx_sb = smalls.tile([1, H2], F32, tag="ctx")
            for dq in range(DQ):
                ctx_ps = cps.tile([1, 512], F32, tag="cps")
                for sb in range(SB):
                    nc.tensor.matmul(
                        ctx_ps[:],
                        lhsT=pt_sb[:, sb:sb + 1],
                        rhs=xn_sb[:, sb, dq * 512:(dq + 1) * 512],
                        start=(sb == 0), stop=(sb == SB - 1),
                    )
                if dq % 2 == 0:
                    nc.vector.tensor_copy(out=ctx_sb[0:1, dq * 512:(dq + 1) * 512], in_=ctx_ps[:])
                else:
                    nc.scalar.copy(out=ctx_sb[0:1, dq * 512:(dq + 1) * 512], in_=ctx_ps[:])
            nc.sync.dma_start(out=ctx_out[i], in_=ctx_sb[0:1, :])

        def emit_scores(i, t_sb):
            # scores[s] = sum_o w_v[o] t[o, s] -> 4 partial rows (PE column
            # groups run concurrently; tile_position derives from the slices)
            s_ps = sps.tile([128, CHUNK], F32, tag="s", bufs=1)
            for r in range(OB // 4):
                for j in range(4):
                    ob = r * 4 + j
                    nc.tensor.matmul(
                        s_ps[32 * j:32 * j + 1, :],
                        lhsT=wv_sb[:, ob:ob + 1],
                        rhs=t_sb[:, ob, :],
                        start=(r == 0), stop=(r == OB // 4 - 1),
                        tile_position=(0, 32 * j),
                    )
            return s_ps

        def emit_softmax(i, s_ps):
            # masked softmax partials: fold the 4 partial rows + mask
            # (DVE may read at most one PSUM operand per op -> serial chain)
            acc_sb = []
            for j in range(4):
                prev = mask_sb[0:1, i, :] if j == 0 else acc_sb[-1][:]
                a = smalls.tile([1, CHUNK], F32, tag=f"fold{j}")
                nc.vector.tensor_tensor(
                    out=a[:], in0=s_ps[32 * j:32 * j + 1, :], in1=prev,
                    op=mybir.AluOpType.add,
                )
                acc_sb.append(a)
            sc_sb = acc_sb[-1]
            negm_sb = smalls.tile([1, 1], F32, tag="negm")
            nc.vector.tensor_reduce(
                out=negm_sb[:], in_=sc_sb[:],
                axis=mybir.AxisListType.X, op=mybir.AluOpType.max, negate=True,
            )
            p_sb = smalls.tile([1, CHUNK], F16, tag="p")
            z_sb = smalls.tile([1, 1], F32, tag="z")
            nc.scalar.activation(
                out=p_sb[:], in_=sc_sb[:],
                func=mybir.ActivationFunctionType.Exp,
                bias=negm_sb[0:1, :], scale=1.0, accum_out=z_sb[:],
            )
            nc.vector.tensor_copy(out=mz_all[0:1, i, 0:1], in_=negm_sb[:])
            nc.vector.tensor_copy(out=mz_all[0:1, i, 1:2], in_=z_sb[:])
            xn_sb = xnp.tile([128, SB, H2], F16, tag="xn")
            return [i, p_sb, xn_sb]

        def emit_pt(p):
            # p row -> column layout [128, SB] via PE transpose.  Deferred to
            # the NEXT group's PE stream (after its e-groups) so the transpose
            # never waits on the softmax chain.
            i, p_sb, xn_sb = p[0], p[1], p[2]
            pt_sb = smalls.tile([128, SB], F16, tag="pt")
            for sb in range(SB):
                t_ps = sps.tile([128, 1], F16, tag="tp", bufs=2)
                nc.tensor.transpose(
                    t_ps[:], p_sb[0:1, sb * 128:(sb + 1) * 128], ident_sb[:])
                nc.vector.tensor_copy(out=pt_sb[:, sb:sb + 1], in_=t_ps[:])
            p[1] = pt_sb

        pending = []
        for g, (k, chunks) in enumerate(groups):
            W = CHUNK * len(chunks)
            if g == 0:
                xt_sb = xt0_sb
            else:
                xt_sb = xtp.tile([128, FB2, 2, W], F8, tag="xt")
                src = xtp_ext[k] if len(chunks) == 2 else xts_ext[0]
                nc.sync.dma_start(out=xt_sb[:], in_=src)
            for p in pending:
                emit_xn_dma(p)  # queued behind this group's xt

            t_list = [tp.tile([128, OB, CHUNK], F16, tag="t", name=f"t{g}_{h}")
                      for h in range(len(chunks))]
            for ob in range(OB):
                if c_sb[ob] is None:
                    emit_c(ob)
                e_ps = eps.tile([128, 2 * CHUNK], F32, tag="e")
                for fb in range(FB2):
                    nc.tensor.matmul(
                        e_ps[:, :W],
                        lhsT=wet_sb[ob][:, fb, :, :],
                        rhs=xt_sb[:, fb, :, :],
                        start=(fb == 0), stop=(fb == FB2 - 1),
                        perf_mode=DR,
                    )
                for h, i in enumerate(chunks):
                    nc.scalar.activation(
                        out=t_list[h][:, ob, :],
                        in_=e_ps[:, h * CHUNK:(h + 1) * CHUNK],
                        func=mybir.ActivationFunctionType.Tanh,
                        bias=c_sb[ob][:, i:i + 1], scale=ISCL,
                    )

            for h, i in enumerate(chunks):
                # drain only chunks from a PREVIOUS group so pt/ctx never
                # wait on a softmax chain that just issued
                drain = bool(pending) and pending[0][3] < g
                if drain:
                    emit_pt(pending[0])
                s_ps = emit_scores(i, t_list[h])
                if drain:
                    emit_ctx(pending.pop(0))
                pending.append(emit_softmax(i, s_ps) + [g, False])

        for p in pending:
            emit_xn_dma(p)
        for p in pending:
            emit_pt(p)
        while pending:
            emit_ctx(pending.pop(0))
        nc.sync.dma_start(out=mz_out[:], in_=mz_all[0:1, :, :])

    nc.compile()
    return nc


def kernel(encoder_out, hidden, W_attn, b_attn, w_v, b_v, lengths):
    encoder_out = np.asarray(encoder_out)
    hidden = np.asarray(hidden)
    W_attn = np.asarray(W_attn)
    b_attn = np.asarray(b_attn)
    w_v = np.asarray(w_v)
    b_v = np.asarray(b_v)
    lengths = np.asarray(lengths)

    # ---- host-side work-unit schedule from the runtime lengths ----
    units = []  # (batch, s0, valid)
    for b in range(B):
        L = int(lengths[b])
        for s0 in range(0, L, CHUNK):
            units.append((b, s0, min(CHUNK, L - s0)))
    nchunk = max(2, (len(units) + N_CORES - 1) // N_CORES)
    npairs = nchunk // 2
    nsingle = nchunk % 2

    We = W_attn[:, H:]                          # [2H, 2H]
    Wh = W_attn[:, :H]                          # [2H, H]

    # ---- fp8 weight quantization (+ residual for the score correction) ----
    W8q = (We * ALPHA).astype(NP_F8)            # [o, f] fp8 payload
    W8f = W8q.astype(np.float32)
    dW = We * ALPHA - W8f                       # exact residual (host)

    # wet[ob, p, fb, i, q] = W8[ob*128+q, fb*256 + i*128 + p]
    wet = np.ascontiguousarray(
        W8q.reshape(OB, 128, FB2, 2, 128).transpose(0, 4, 2, 3, 1))
    # wht[ob, p, jh, q]: blocks 0..HB-1 of W_h^T; an extra block whose row
    # p=0 carries b_attn is appended only when b_attn is nonzero
    nhb = HB + 1 if np.any(b_attn) else HB
    wht_aug = np.zeros((nhb * 128, H2), np.float32)
    wht_aug[:H] = Wh.T
    if nhb > HB:
        wht_aug[H] = b_attn
    wht = np.ascontiguousarray(
        wht_aug.reshape(nhb, 128, OB, 128).transpose(2, 1, 0, 3)
    ).astype(np.float16)
    wv = np.ascontiguousarray(w_v[0].reshape(OB, 128).T).astype(np.float16)

    # ---- linearized fp8 score correction (host, exact residual algebra) ----
    # kappa[o, b] = E_s[1 - tanh^2(e8[o,s])] with e8 ~ N(c[o,b], sigma_o^2)
    c_all = (Wh @ hidden) + b_attn[:, None]     # [2H, B]
    sig = np.linalg.norm(We, axis=1)            # [2H]
    gh_x, gh_w = np.polynomial.hermite_e.hermegauss(8)
    gh_w = (gh_w / gh_w.sum()).astype(np.float64)
    z = c_all[:, None, :] + sig[:, None, None] * gh_x[None, :, None]
    kappa = np.einsum("okb,k->ob", 1.0 / np.cosh(z) ** 2, gh_w,
                      optimize=True).astype(np.float32)   # [2H, B]
    wk = w_v[0][:, None] * kappa                # [2H_o, B]
    g_all = dW.T @ wk                           # [2H_f, B]
    h8_all = W8f.T @ wk                         # [2H_f, B]

    # ---- per-core gathered inputs ----
    in_maps = []
    slot_of = []  # per real unit: (core, slot)
    x16 = encoder_out.astype(np.float16)
    x8 = np.empty((B, S, H2), NP_F8)
    corr = np.empty((B, S), np.float32)
    for b in range(B):
        x8[b] = (encoder_out[b] * BETA).astype(NP_F8)
        x8f = x8[b].astype(np.float32)          # [S, 2H]
        dX = encoder_out[b] * BETA - x8f
        corr[b] = (BETA * (encoder_out[b] @ g_all[:, b])
                   + dX @ h8_all[:, b]) * ISCL

    nc = build_program(nchunk, nhb)

    for c in range(N_CORES):
        cu = units[c * nchunk:(c + 1) * nchunk]
        xtp_a = np.zeros((max(1, npairs), 128, FB2, 2, 2 * CHUNK), NP_F8)
        xts_a = np.zeros((1, 128, FB2, 2, CHUNK), NP_F8)
        xn = np.zeros((nchunk, 128, CHUNK // 128, H2), np.float16)
        mask = np.full((nchunk, CHUNK), NEG + float(b_v[0]), np.float32)
        hu = np.zeros((128, nhb, nchunk), np.float16)
        if nhb > HB:
            hu[0, HB, :] = 1.0
        for slot, (b, s0, v) in enumerate(cu):
            chunk8 = x8[b, s0:s0 + v, :]                     # [v, 2048] fp8
            # xt[p, fb, i, s] = x8[s, fb*256 + i*128 + p]
            xt_block = np.zeros((128, FB2, 2, CHUNK), NP_F8)
            xt_block[:, :, :, :v] = chunk8.reshape(v, FB2, 2, 128).transpose(3, 1, 2, 0)
            if slot // 2 < npairs:
                half = slot % 2
                xtp_a[slot // 2, :, :, :, half * CHUNK:(half + 1) * CHUNK] = xt_block
            else:
                xts_a[0] = xt_block
            # xn[slot, p, sb, d] = x16[sb*128 + p, d]
            full = np.zeros((CHUNK, H2), np.float16)
            full[:v] = x16[b, s0:s0 + v, :]
            xn[slot] = full.reshape(CHUNK // 128, 128, H2).transpose(1, 0, 2)
            mask[slot, :v] = float(b_v[0]) + corr[b, s0:s0 + v]
            hu[:, :HB, slot] = hidden[:, b].reshape(HB, 128).T
            slot_of.append((c, slot))
        m = dict(xn=xn, mask=mask, hu=hu, wet=wet, wht=wht, wv=wv)
        if npairs:
            m["xtp"] = xtp_a
        if nsingle:
            m["xts"] = xts_a
        in_maps.append(m)

    def run_once():
        res = run_bass_kernel_spmd(nc, in_maps, core_ids=list(range(N_CORES)))
        negm = np.stack([res.results[c]["out_mz"][:, 0] for c in range(N_CORES)])
        zz = np.stack([res.results[c]["out_mz"][:, 1] for c in range(N_CORES)])
        ctx = np.stack([res.results[c]["out_ctx"] for c in range(N_CORES)])
        return negm, zz, ctx

    def merge(parts):
        negm, zz, ctx = parts
        # ---- exact flash-softmax merge on host ----
        out = np.zeros((B, H2), np.float32)
        ok = np.isfinite(negm).all() and np.isfinite(zz).all() and np.isfinite(ctx).all()
        for b in range(B):
            idxs = [slot_of[k] for k, (ub, _, _) in enumerate(units) if ub == b]
            ms = np.array([-float(negm[c, s]) for c, s in idxs])
            m = ms.max()
            w = np.exp(ms - m)
            Z = float(sum(wi * float(zz[c, s]) for wi, (c, s) in zip(w, idxs)))
            if not (Z > 0):
                ok = False
                Z = 1.0
            acc = np.zeros(H2, np.float64)
            for wi, (c, s) in zip(w, idxs):
                acc += wi * ctx[c, s].astype(np.float64)
            out[b] = (acc / Z).astype(np.float32)
        # context rows are convex combinations of encoder_out rows
        ok = ok and np.isfinite(out).all() and np.abs(out).max() < 50.0
        return out, ok

    out, ok = merge(run_once())
    if not ok:  # one retry on gross corruption
        out, ok = merge(run_once())
    return out
